# revision 22
# baseline (speedup 1.0000x reference)
"""Crystal segment-norm kernel for 8 Trainium2 NeuronCores.

Transposed-layout strategy (features on partitions, atoms along free dim):
- Host packs whole segments into fixed 8192-atom chunks (pad each segment
  to a multiple of G=8 atoms; <=127 segments per chunk, slot 127 = trash),
  then ships each chunk TRANSPOSED and j-interleaved as fp16:
  xT[c, f, j*1024 + g] = xpad[c*8192 + 8*g + j, f].
- Per chunk the engines split the work so each stays under the DMA time:
  Scalar squares x (fp16). GPSIMD avg-pools x^2 over the 8 j-positions
  (group mean; the x8 is folded into the stats). DVE does a 3-level
  pairwise tensor_tensor tree for group sums of x (fp16 at the 2x rate).
  PE transposes the [f, group] sums to group-major, matmuls with
  at[g,s]=(gseg[g]==s) to get per-segment sums in PSUM, stats math makes
  K = w/(std+eps), C = b - mean*K as [s, f] fp16, and PE expands them
  back to per-group [f, 1024] via Ag = at^T. Normalize is two DVE
  tensor_tensor passes with a stride-0 broadcast of per-group K and C
  over the 8 j-positions: out = x*Kb + Cb, all fp16.
- Chunks are software-pipelined: pass1 for chunk c is emitted alongside
  stats for c-1 and normalize for c-2, so no engine waits in steady state.
"""
import numpy as np

N = 1_000_000
F = 128
S = 16_384
EPS = 1e-6
VAR_FLOOR = 1e-7
NCORES = 8
G = 8                      # segment atom-count padding granularity
P = 128
NG_C = 1024                # groups per chunk
NSL = 8                    # group slices (of 128) per chunk
CHUNK_ATOMS = NG_C * G     # 8192
MAXSEG = 127               # real segments per chunk; slot 127 = trash
TRASH = 127


def _plan(index):
    """Pack segments into per-core chunk layouts. Returns per-core plans."""
    counts = np.bincount(index, minlength=S).astype(np.int64)
    seg_start = np.concatenate([[0], np.cumsum(counts)[:-1]])
    pad = ((counts + G - 1) // G) * G
    csum = np.cumsum(pad)
    total = int(csum[-1])
    bounds = [0]
    for c in range(1, NCORES):
        bounds.append(int(np.searchsorted(csum, total * c / NCORES)))
    bounds.append(S)

    plans = []
    for c in range(NCORES):
        segs = [s for s in range(bounds[c], bounds[c + 1]) if counts[s] > 0]
        chunks = []
        cur, cur_atoms = [], 0
        for s in segs:
            p = int(pad[s])
            assert p <= CHUNK_ATOMS
            if cur_atoms + p > CHUNK_ATOMS or len(cur) >= MAXSEG:
                chunks.append(cur)
                cur, cur_atoms = [], 0
            cur.append(s)
            cur_atoms += p
        if cur:
            chunks.append(cur)
        plans.append((chunks, counts, seg_start, pad))
    return plans


def _core_arrays(plan, nchunks, xh):
    """Build xT/gsegc/rn/rn1 + row maps for one core."""
    chunks, counts, seg_start, pad = plan
    nat = nchunks * CHUNK_ATOMS
    ngrp = nat // G
    gseg = np.full(ngrp, TRASH, dtype=np.float32)
    rn = np.ones((nchunks, P), dtype=np.float32)
    rn1 = np.ones((nchunks, P), dtype=np.float32)

    seg_n, seg_src, seg_dst = [], [], []
    n1_dst = []  # dst rows of n==1 segments (host post-fix)
    for ci, segs in enumerate(chunks):
        off = ci * CHUNK_ATOMS
        for l, s in enumerate(segs):
            n = int(counts[s])
            p = int(pad[s])
            seg_n.append(n)
            seg_src.append(int(seg_start[s]))
            seg_dst.append(off)
            gseg[off // G:(off + p) // G] = l
            rn[ci, l] = 1.0 / n
            rn1[ci, l] = 1.0 / (n - 1) if n > 1 else 1.0
            if n == 1:
                n1_dst.append(off)
            off += p

    seg_n = np.array(seg_n, dtype=np.int64)
    seg_src = np.array(seg_src, dtype=np.int64)
    seg_dst = np.array(seg_dst, dtype=np.int64)
    tot = int(seg_n.sum())
    starts = np.concatenate([[0], np.cumsum(seg_n)[:-1]])
    local = np.arange(tot, dtype=np.int64) - np.repeat(starts, seg_n)
    src_rows = np.repeat(seg_src, seg_n) + local
    dst_rows = np.repeat(seg_dst, seg_n) + local

    xpad = np.zeros((nat, F), dtype=np.float16)
    xpad[dst_rows] = xh[src_rows]
    # [c, f, j, g] layout, contiguous
    xT = np.ascontiguousarray(
        xpad.reshape(nchunks, NG_C, G, F).transpose(0, 3, 2, 1)
    ).reshape(nchunks, F, CHUNK_ATOMS)
    gsegc = np.ascontiguousarray(
        gseg.reshape(nchunks, NSL, P).transpose(0, 2, 1))
    # at[c, g, k*128+s] = (gseg[c*1024 + k*128 + g] == s), fp16
    at = (gsegc[:, :, :, None] ==
          np.arange(P, dtype=np.float32)[None, None, None, :])
    at = at.astype(np.float16).reshape(nchunks, P, NSL * P)
    return {
        "xT": xT,
        "gsegc": gsegc,
        "at": at,
        "rn": rn.reshape(nchunks * P, 1),
        "rn1": rn1.reshape(nchunks * P, 1),
        "src_rows": src_rows,
        "dst_rows": dst_rows,
        "n1_dst": np.array(n1_dst, dtype=np.int64),
    }


def _consts(weight, bias):
    return {
        "ident_h": np.eye(P, dtype=np.float16),
        "wb": np.tile(np.asarray(weight, dtype=np.float32), (P, 1)),
        "bb": np.tile(np.asarray(bias, dtype=np.float32), (P, 1)),
    }


def _build(nchunks, fastwb):
    import concourse.tile as tile
    from concourse import bacc, mybir

    F32 = mybir.dt.float32
    FP16 = mybir.dt.float16
    AF = mybir.ActivationFunctionType
    OP = mybir.AluOpType

    NA = CHUNK_ATOMS
    nc = bacc.Bacc("TRN2", target_bir_lowering=False, debug=False,
                   num_devices=NCORES)
    xT_d = nc.dram_tensor("xT", [nchunks, P, NA], FP16, kind="ExternalInput")
    out_d = nc.dram_tensor("out", [nchunks, P, NA], FP16,
                           kind="ExternalOutput")
    at_d = nc.dram_tensor("at", [nchunks, P, NSL * P], FP16,
                          kind="ExternalInput")
    rn_d = nc.dram_tensor("rn", [nchunks * P, 1], F32, kind="ExternalInput")
    rn1_d = nc.dram_tensor("rn1", [nchunks * P, 1], F32, kind="ExternalInput")
    ident_d = nc.dram_tensor("ident_h", [P, P], FP16, kind="ExternalInput")
    wb_d = nc.dram_tensor("wb", [P, P], F32, kind="ExternalInput")
    bb_d = nc.dram_tensor("bb", [P, P], F32, kind="ExternalInput")

    with tile.TileContext(nc) as tc:
        with (
            tc.tile_pool(name="consts", bufs=1) as cpool,
            tc.tile_pool(name="xp", bufs=5) as xp,
            tc.tile_pool(name="sqp", bufs=1) as sqp,
            tc.tile_pool(name="treep", bufs=1) as treep,
            tc.tile_pool(name="gsump", bufs=3) as gsump,
            tc.tile_pool(name="gtp", bufs=2) as gtp,
            tc.tile_pool(name="atp", bufs=3) as atp,
            tc.tile_pool(name="agp", bufs=2) as agp,
            tc.tile_pool(name="kcp", bufs=3) as kcp,
            tc.tile_pool(name="statp", bufs=2) as statp,
            tc.tile_pool(name="smallp", bufs=2) as smallp,
            tc.tile_pool(name="op_", bufs=2) as op_,
            tc.tile_pool(name="ps_tr", bufs=1, space="PSUM") as ps_tr,
            tc.tile_pool(name="ps_seg", bufs=2, space="PSUM") as ps_seg,
            tc.tile_pool(name="ps_ag", bufs=1, space="PSUM") as ps_ag,
            tc.tile_pool(name="ps_kg", bufs=1, space="PSUM") as ps_kg,
        ):
            ident_t = cpool.tile([P, P], FP16)
            nc.sync.dma_start(out=ident_t[:], in_=ident_d.ap()[:, :])
            wb_t = cpool.tile([P, P], F32)
            nc.sync.dma_start(out=wb_t[:], in_=wb_d.ap()[:, :])
            bb_t = cpool.tile([P, P], F32)
            nc.sync.dma_start(out=bb_t[:], in_=bb_d.ap()[:, :])

            def load(c):
                x_t = xp.tile([P, NA], FP16, tag="x")
                nc.sync.dma_start(out=x_t[:], in_=xT_d.ap()[c, :, :])
                at_t = atp.tile([P, NSL * P], FP16, tag="at")
                nc.sync.dma_start(out=at_t[:], in_=at_d.ap()[c, :, :])
                return x_t, at_t

            def pass1a(c, x_t):
                """Square (ACT); x-tree l1 (DVE), l2/l3 (GPSIMD)."""
                xsq = sqp.tile([P, NA], FP16, tag="xsq")
                nc.scalar.activation(out=xsq[:], in_=x_t[:], func=AF.Square)

                g1 = treep.tile([P, NA // 2], FP16, tag="g1")
                nc.vector.tensor_tensor(
                    out=g1[:], in0=x_t[:, 0:NA // 2], in1=x_t[:, NA // 2:NA],
                    op=OP.add)
                g2 = treep.tile([P, NA // 4], FP16, tag="g2")
                nc.gpsimd.tensor_tensor(
                    out=g2[:], in0=g1[:, 0:NA // 4], in1=g1[:, NA // 4:NA // 2],
                    op=OP.add)
                gsum = gsump.tile([P, NG_C], FP16, tag="gsum")
                nc.gpsimd.tensor_tensor(
                    out=gsum[:], in0=g2[:, 0:NA // 8], in1=g2[:, NA // 8:NA // 4],
                    op=OP.add)
                return xsq, gsum

            def pass1b(c, xsq):
                """xsq-tree: l1/l2 on DVE (fp16 2x), l3 on GPSIMD."""
                l1 = treep.tile([P, NA // 2], FP16, tag="l1")
                nc.vector.tensor_tensor(
                    out=l1[:], in0=xsq[:, 0:NA // 2], in1=xsq[:, NA // 2:NA],
                    op=OP.add)
                l2 = treep.tile([P, NA // 4], FP16, tag="l2")
                nc.vector.tensor_tensor(
                    out=l2[:], in0=l1[:, 0:NA // 4], in1=l1[:, NA // 4:NA // 2],
                    op=OP.add)
                gsq = gsump.tile([P, NG_C], FP16, tag="gsq")
                nc.vector.tensor_tensor(
                    out=gsq[:], in0=l2[:, 0:NA // 8], in1=l2[:, NA // 8:NA // 4],
                    op=OP.add)
                return gsq

            def pass2(c, gsum, gsq, at_t):
                """Transposes + agg (PE), gT/ag copies (ACT) for chunk c."""
                tr_ps = ps_tr.tile([P, 2 * NSL * P], FP16, space="PSUM",
                                   tag="tr")
                for k in range(NSL):
                    nc.tensor.transpose(
                        out=tr_ps[:, k * 2 * P:k * 2 * P + P],
                        in_=gsum[:, k * P:(k + 1) * P], identity=ident_t[:])
                    nc.tensor.transpose(
                        out=tr_ps[:, k * 2 * P + P:(k + 1) * 2 * P],
                        in_=gsq[:, k * P:(k + 1) * P], identity=ident_t[:])
                ag_ps = ps_ag.tile([P, NSL * P], FP16, space="PSUM", tag="ag")
                for k in range(NSL):
                    nc.tensor.transpose(
                        out=ag_ps[:, k * P:(k + 1) * P],
                        in_=at_t[:, k * P:(k + 1) * P], identity=ident_t[:])
                gT = gtp.tile([P, 2 * NSL * P], FP16, tag="gT")
                nc.scalar.copy(out=gT[:], in_=tr_ps[:])
                ag_t = agp.tile([P, NSL * P], FP16, tag="ag")
                nc.scalar.copy(out=ag_t[:], in_=ag_ps[:])

                seg_ps = ps_seg.tile([P, 2 * P], F32, space="PSUM", tag="seg")
                for k in range(NSL):
                    nc.tensor.matmul(
                        out=seg_ps[:],
                        lhsT=at_t[:, k * P:(k + 1) * P],
                        rhs=gT[:, k * 2 * P:(k + 1) * 2 * P],
                        start=(k == 0), stop=(k == NSL - 1),
                    )
                return seg_ps, ag_t

            def stage2(c, seg_ps, ag_t):
                """Stats for chunk c -> expand K,C to groups -> kcg sbuf."""
                rn_t = smallp.tile([P, 1], F32, tag="rn")
                nc.sync.dma_start(out=rn_t[:],
                                  in_=rn_d.ap()[c * P:(c + 1) * P, :])
                rn1_t = smallp.tile([P, 1], F32, tag="rn1")
                nc.sync.dma_start(out=rn1_t[:],
                                  in_=rn1_d.ap()[c * P:(c + 1) * P, :])
                mean_t = statp.tile([P, P], F32, tag="mean")
                nc.scalar.activation(
                    out=mean_t[:], in_=seg_ps[:, 0:P], func=AF.Copy,
                    scale=rn_t[:],
                )
                t1 = statp.tile([P, P], F32, tag="t1")
                nc.vector.tensor_tensor(
                    out=t1[:], in0=mean_t[:], in1=seg_ps[:, 0:P], op=OP.mult)
                var_t = statp.tile([P, P], F32, tag="var")
                nc.vector.tensor_tensor(
                    out=var_t[:], in0=seg_ps[:, P:2 * P], in1=t1[:],
                    op=OP.subtract)
                nc.vector.tensor_scalar(
                    out=var_t[:], in0=var_t[:], scalar1=float(EPS),
                    scalar2=rn1_t[:], op0=OP.add, op1=OP.mult,
                )
                nc.vector.tensor_scalar(
                    out=var_t[:], in0=var_t[:], scalar1=float(VAR_FLOOR),
                    scalar2=None, op0=OP.max,
                )
                std_t = statp.tile([P, P], F32, tag="std")
                nc.scalar.activation(out=std_t[:], in_=var_t[:], func=AF.Sqrt)
                nc.scalar.activation(out=std_t[:], in_=std_t[:], func=AF.Copy,
                                     bias=float(EPS))
                kc32 = statp.tile([P, 2 * P], F32, tag="kc32")
                if fastwb:
                    nc.vector.reciprocal(out=kc32[:, 0:P], in_=std_t[:])
                    mk_t = statp.tile([P, P], F32, tag="mk")
                    nc.vector.tensor_tensor(
                        out=mk_t[:], in0=mean_t[:], in1=kc32[:, 0:P],
                        op=OP.mult)
                    nc.vector.tensor_scalar(
                        out=kc32[:, P:2 * P], in0=mk_t[:], scalar1=-1.0,
                        scalar2=None, op0=OP.mult)
                else:
                    rstd_t = statp.tile([P, P], F32, tag="rstd")
                    nc.vector.reciprocal(out=rstd_t[:], in_=std_t[:])
                    nc.vector.tensor_tensor(
                        out=kc32[:, 0:P], in0=rstd_t[:], in1=wb_t[:],
                        op=OP.mult)
                    mk_t = statp.tile([P, P], F32, tag="mk")
                    nc.vector.tensor_tensor(
                        out=mk_t[:], in0=mean_t[:], in1=kc32[:, 0:P],
                        op=OP.mult)
                    nc.vector.tensor_tensor(
                        out=kc32[:, P:2 * P], in0=bb_t[:], in1=mk_t[:],
                        op=OP.subtract)
                kc16 = statp.tile([P, 2 * P], FP16, tag="kc16")
                nc.scalar.copy(out=kc16[:], in_=kc32[:])

                kcg = kcp.tile([P, 2 * NG_C], FP16, tag="kcg")
                for half in range(2):
                    kg_ps = ps_kg.tile([P, NG_C], F32, space="PSUM", tag="kg")
                    src_t = kc16[:, half * P:(half + 1) * P]
                    for h in range(2):
                        nc.tensor.matmul(
                            out=kg_ps[:, h * 512:(h + 1) * 512],
                            lhsT=src_t, rhs=ag_t[:, h * 512:(h + 1) * 512],
                            start=True, stop=True,
                        )
                    nc.scalar.copy(
                        out=kcg[:, half * NG_C:(half + 1) * NG_C],
                        in_=kg_ps[:])
                return kcg

            def stage3(c, x_t, kcg):
                """Normalize chunk c: out = x*Kb + Cb, store."""
                x3 = x_t[:].rearrange("p (j g) -> p j g", g=NG_C)
                kb = kcg[:, 0:NG_C].rearrange("p (o g) -> p o g", o=1)
                kb = kb.broadcast_to([P, G, NG_C])
                cb = kcg[:, NG_C:2 * NG_C].rearrange("p (o g) -> p o g", o=1)
                cb = cb.broadcast_to([P, G, NG_C])
                y_t = sqp.tile([P, NA], FP16, tag="xsq")
                y3 = y_t[:].rearrange("p (j g) -> p j g", g=NG_C)
                nc.vector.tensor_tensor(out=y3, in0=x3, in1=kb, op=OP.mult)
                o_t = op_.tile([P, NA], FP16, tag="o")
                o3 = o_t[:].rearrange("p (j g) -> p j g", g=NG_C)
                nc.vector.tensor_tensor(
                    out=o3[:, 0:6, :], in0=y3[:, 0:6, :],
                    in1=cb[:, 0:6, :], op=OP.add)
                nc.gpsimd.tensor_tensor(
                    out=o3[:, 6:8, :], in0=y3[:, 6:8, :],
                    in1=cb[:, 6:8, :], op=OP.add)
                nc.scalar.dma_start(out=out_d.ap()[c, :, :], in_=o_t[:])

            loaded = {0: load(0)}
            p1 = {}     # c -> (x_t, at_t, xsq, gsum)
            p3 = {}     # c -> (x_t, kcg)

            def mid(c):
                # pass2 + stats for chunk c (emitted at iter c+2)
                x_t, at_t, xsq, gsum = p1.pop(c)
                gsq = p1gsq.pop(c)
                seg_ps, ag_t = pass2(c, gsum, gsq, at_t)
                kcg = stage2(c, seg_ps, ag_t)
                p3[c] = (x_t, kcg)

            p1gsq = {}
            for c in range(nchunks):
                if c + 1 < nchunks:
                    loaded[c + 1] = load(c + 1)
                if c >= 3:
                    x_t, kcg = p3.pop(c - 3)
                    stage3(c - 3, x_t, kcg)
                x_t, at_t = loaded.pop(c)
                xsq, gsum = pass1a(c, x_t)
                p1[c] = (x_t, at_t, xsq, gsum)
                if c >= 2:
                    mid(c - 2)
                p1gsq[c] = pass1b(c, xsq)
            for c in range(nchunks - 2, nchunks):
                mid(c)
            for c in range(nchunks - 3, nchunks):
                if c >= 0:
                    x_t, kcg = p3.pop(c)
                    stage3(c, x_t, kcg)

    nc.compile()
    return nc


_BUILD_CACHE = {}


def kernel(target_fea, index, weight, bias):
    from concourse.bass_utils import run_bass_kernel_spmd

    x = np.asarray(target_fea, dtype=np.float32)
    idx = np.asarray(index, dtype=np.int64)
    xh = x.astype(np.float16)
    plans = _plan(idx)
    nchunks = max(len(p[0]) for p in plans)
    consts = _consts(weight, bias)

    cores = [_core_arrays(p, nchunks, xh) for p in plans]
    in_maps = []
    for ca in cores:
        m = {"xT": ca["xT"], "at": ca["at"], "rn": ca["rn"],
             "rn1": ca["rn1"]}
        m.update(consts)
        in_maps.append(m)

    fastwb = bool(np.all(np.asarray(weight) == 1.0)
                  and np.all(np.asarray(bias) == 0.0))
    key = (nchunks, fastwb)
    if key not in _BUILD_CACHE:
        _BUILD_CACHE[key] = _build(nchunks, fastwb)
    nc = _BUILD_CACHE[key]

    res = run_bass_kernel_spmd(nc, in_maps, core_ids=list(range(NCORES)))

    out = np.empty((N, F), dtype=np.float32)
    bias_np = np.asarray(bias, dtype=np.float32)
    for c in range(NCORES):
        ca = cores[c]
        r = np.asarray(res.results[c]["out"])
        atoms = np.ascontiguousarray(
            r.reshape(nchunks, F, G, NG_C).transpose(0, 3, 2, 1)
        ).reshape(nchunks * CHUNK_ATOMS, F)
        out[ca["src_rows"]] = atoms[ca["dst_rows"]].astype(np.float32)
        for d in ca["n1_dst"]:
            # n==1 segments: reference yields exactly bias
            src = ca["src_rows"][np.searchsorted(ca["dst_rows"], d)]
            out[src] = bias_np
    return out


# revision 23
# speedup vs baseline: 1.0587x; 1.0587x over previous
"""Crystal segment-norm kernel for 8 Trainium2 NeuronCores.

Transposed-layout strategy (features on partitions, atoms along free dim):
- Host packs whole segments into fixed 8192-atom chunks (pad each segment
  to a multiple of G=8 atoms; <=127 segments per chunk, slot 127 = trash),
  then ships each chunk TRANSPOSED and j-interleaved as fp16:
  xT[c, f, j*1024 + g] = xpad[c*8192 + 8*g + j, f].
- Per chunk the engines split the work so each stays under the DMA time:
  Scalar squares x (fp16). GPSIMD avg-pools x^2 over the 8 j-positions
  (group mean; the x8 is folded into the stats). DVE does a 3-level
  pairwise tensor_tensor tree for group sums of x (fp16 at the 2x rate).
  PE transposes the [f, group] sums to group-major, matmuls with
  at[g,s]=(gseg[g]==s) to get per-segment sums in PSUM, stats math makes
  K = w/(std+eps), C = b - mean*K as [s, f] fp16, and PE expands them
  back to per-group [f, 1024] via Ag = at^T. Normalize is two DVE
  tensor_tensor passes with a stride-0 broadcast of per-group K and C
  over the 8 j-positions: out = x*Kb + Cb, all fp16.
- Chunks are software-pipelined: pass1 for chunk c is emitted alongside
  stats for c-1 and normalize for c-2, so no engine waits in steady state.
"""
import numpy as np

N = 1_000_000
F = 128
S = 16_384
EPS = 1e-6
VAR_FLOOR = 1e-7
NCORES = 8
G = 8                      # segment atom-count padding granularity
P = 128
NG_C = 1024                # groups per chunk
NSL = 8                    # group slices (of 128) per chunk
CHUNK_ATOMS = NG_C * G     # 8192
MAXSEG = 127               # real segments per chunk; slot 127 = trash
TRASH = 127


def _plan(index):
    """Pack segments into per-core chunk layouts. Returns per-core plans."""
    counts = np.bincount(index, minlength=S).astype(np.int64)
    seg_start = np.concatenate([[0], np.cumsum(counts)[:-1]])
    pad = ((counts + G - 1) // G) * G
    csum = np.cumsum(pad)
    total = int(csum[-1])
    bounds = [0]
    for c in range(1, NCORES):
        bounds.append(int(np.searchsorted(csum, total * c / NCORES)))
    bounds.append(S)

    plans = []
    for c in range(NCORES):
        segs = [s for s in range(bounds[c], bounds[c + 1]) if counts[s] > 0]
        chunks = []
        cur, cur_atoms = [], 0
        for s in segs:
            p = int(pad[s])
            assert p <= CHUNK_ATOMS
            if cur_atoms + p > CHUNK_ATOMS or len(cur) >= MAXSEG:
                chunks.append(cur)
                cur, cur_atoms = [], 0
            cur.append(s)
            cur_atoms += p
        if cur:
            chunks.append(cur)
        plans.append((chunks, counts, seg_start, pad))
    return plans


def _core_arrays(plan, nchunks, xh):
    """Build xT/gsegc/rn/rn1 + row maps for one core."""
    chunks, counts, seg_start, pad = plan
    nat = nchunks * CHUNK_ATOMS
    ngrp = nat // G
    gseg = np.full(ngrp, TRASH, dtype=np.float32)
    rn = np.ones((nchunks, P), dtype=np.float32)
    rn1 = np.ones((nchunks, P), dtype=np.float32)

    seg_n, seg_src, seg_dst = [], [], []
    n1_dst = []  # dst rows of n==1 segments (host post-fix)
    for ci, segs in enumerate(chunks):
        off = ci * CHUNK_ATOMS
        for l, s in enumerate(segs):
            n = int(counts[s])
            p = int(pad[s])
            seg_n.append(n)
            seg_src.append(int(seg_start[s]))
            seg_dst.append(off)
            gseg[off // G:(off + p) // G] = l
            rn[ci, l] = 1.0 / n
            rn1[ci, l] = 1.0 / (n - 1) if n > 1 else 1.0
            if n == 1:
                n1_dst.append(off)
            off += p

    seg_n = np.array(seg_n, dtype=np.int64)
    seg_src = np.array(seg_src, dtype=np.int64)
    seg_dst = np.array(seg_dst, dtype=np.int64)
    tot = int(seg_n.sum())
    starts = np.concatenate([[0], np.cumsum(seg_n)[:-1]])
    local = np.arange(tot, dtype=np.int64) - np.repeat(starts, seg_n)
    src_rows = np.repeat(seg_src, seg_n) + local
    dst_rows = np.repeat(seg_dst, seg_n) + local

    xpad = np.zeros((nat, F), dtype=np.float16)
    xpad[dst_rows] = xh[src_rows]
    # [c, f, j, g] layout, contiguous
    xT = np.ascontiguousarray(
        xpad.reshape(nchunks, NG_C, G, F).transpose(0, 3, 2, 1)
    ).reshape(nchunks, F, CHUNK_ATOMS)
    gsegc = np.ascontiguousarray(
        gseg.reshape(nchunks, NSL, P).transpose(0, 2, 1))
    # at[c, g, k*128+s] = (gseg[c*1024 + k*128 + g] == s), fp16
    at = (gsegc[:, :, :, None] ==
          np.arange(P, dtype=np.float32)[None, None, None, :])
    at = at.astype(np.float16).reshape(nchunks, P, NSL * P)
    return {
        "xT": xT,
        "gsegc": gsegc,
        "at": at,
        "rn": rn.reshape(nchunks * P, 1),
        "rn1": rn1.reshape(nchunks * P, 1),
        "src_rows": src_rows,
        "dst_rows": dst_rows,
        "n1_dst": np.array(n1_dst, dtype=np.int64),
    }


def _consts(weight, bias):
    return {
        "ident_h": np.eye(P, dtype=np.float16),
        "wb": np.tile(np.asarray(weight, dtype=np.float32), (P, 1)),
        "bb": np.tile(np.asarray(bias, dtype=np.float32), (P, 1)),
    }


def _build(nchunks, fastwb):
    import concourse.tile as tile
    from concourse import bacc, mybir

    F32 = mybir.dt.float32
    FP16 = mybir.dt.float16
    AF = mybir.ActivationFunctionType
    OP = mybir.AluOpType

    NA = CHUNK_ATOMS
    nc = bacc.Bacc("TRN2", target_bir_lowering=False, debug=False,
                   num_devices=NCORES)
    xT_d = nc.dram_tensor("xT", [nchunks, P, NA], FP16, kind="ExternalInput")
    out_d = nc.dram_tensor("out", [nchunks, P, NA], FP16,
                           kind="ExternalOutput")
    at_d = nc.dram_tensor("at", [nchunks, P, NSL * P], FP16,
                          kind="ExternalInput")
    rn_d = nc.dram_tensor("rn", [nchunks * P, 1], F32, kind="ExternalInput")
    rn1_d = nc.dram_tensor("rn1", [nchunks * P, 1], F32, kind="ExternalInput")
    ident_d = nc.dram_tensor("ident_h", [P, P], FP16, kind="ExternalInput")
    wb_d = nc.dram_tensor("wb", [P, P], F32, kind="ExternalInput")
    bb_d = nc.dram_tensor("bb", [P, P], F32, kind="ExternalInput")

    with tile.TileContext(nc) as tc:
        with (
            tc.tile_pool(name="consts", bufs=1) as cpool,
            tc.tile_pool(name="xp", bufs=5) as xp,
            tc.tile_pool(name="sqp", bufs=1) as sqp,
            tc.tile_pool(name="treep", bufs=1) as treep,
            tc.tile_pool(name="gsump", bufs=3) as gsump,
            tc.tile_pool(name="gtp", bufs=2) as gtp,
            tc.tile_pool(name="atp", bufs=3) as atp,
            tc.tile_pool(name="agp", bufs=2) as agp,
            tc.tile_pool(name="kcp", bufs=3) as kcp,
            tc.tile_pool(name="statp", bufs=2) as statp,
            tc.tile_pool(name="smallp", bufs=2) as smallp,
            tc.tile_pool(name="op_", bufs=2) as op_,
            tc.tile_pool(name="ps_tr", bufs=1, space="PSUM") as ps_tr,
            tc.tile_pool(name="ps_seg", bufs=2, space="PSUM") as ps_seg,
            tc.tile_pool(name="ps_ag", bufs=1, space="PSUM") as ps_ag,
            tc.tile_pool(name="ps_kg", bufs=1, space="PSUM") as ps_kg,
        ):
            ident_t = cpool.tile([P, P], FP16)
            nc.sync.dma_start(out=ident_t[:], in_=ident_d.ap()[:, :])
            wb_t = cpool.tile([P, P], F32)
            nc.sync.dma_start(out=wb_t[:], in_=wb_d.ap()[:, :])
            bb_t = cpool.tile([P, P], F32)
            nc.sync.dma_start(out=bb_t[:], in_=bb_d.ap()[:, :])

            def load(c):
                x_t = xp.tile([P, NA], FP16, tag="x")
                nc.sync.dma_start(out=x_t[:], in_=xT_d.ap()[c, :, :])
                at_t = atp.tile([P, NSL * P], FP16, tag="at")
                nc.sync.dma_start(out=at_t[:], in_=at_d.ap()[c, :, :])
                return x_t, at_t

            def pass1a(c, x_t):
                """Square (ACT); x-tree l1 (DVE), l2/l3 (GPSIMD)."""
                xsq = sqp.tile([P, NA], FP16, tag="xsq")
                nc.scalar.activation(out=xsq[:], in_=x_t[:], func=AF.Square)

                g1 = treep.tile([P, NA // 2], FP16, tag="g1")
                nc.vector.tensor_tensor(
                    out=g1[:], in0=x_t[:, 0:NA // 2], in1=x_t[:, NA // 2:NA],
                    op=OP.add)
                g2 = treep.tile([P, NA // 4], FP16, tag="g2")
                nc.gpsimd.tensor_tensor(
                    out=g2[:], in0=g1[:, 0:NA // 4], in1=g1[:, NA // 4:NA // 2],
                    op=OP.add)
                gsum = gsump.tile([P, NG_C], FP16, tag="gsum")
                nc.gpsimd.tensor_tensor(
                    out=gsum[:], in0=g2[:, 0:NA // 8], in1=g2[:, NA // 8:NA // 4],
                    op=OP.add)
                return xsq, gsum

            def pass1b(c, xsq):
                """xsq-tree: l1/l2 on DVE (fp16 2x), l3 on GPSIMD."""
                l1 = treep.tile([P, NA // 2], FP16, tag="l1")
                nc.vector.tensor_tensor(
                    out=l1[:], in0=xsq[:, 0:NA // 2], in1=xsq[:, NA // 2:NA],
                    op=OP.add)
                l2 = treep.tile([P, NA // 4], FP16, tag="l2")
                nc.vector.tensor_tensor(
                    out=l2[:], in0=l1[:, 0:NA // 4], in1=l1[:, NA // 4:NA // 2],
                    op=OP.add)
                gsq = gsump.tile([P, NG_C], FP16, tag="gsq")
                nc.vector.tensor_tensor(
                    out=gsq[:], in0=l2[:, 0:NA // 8], in1=l2[:, NA // 8:NA // 4],
                    op=OP.add)
                return gsq

            def pass2(c, gsum, gsq, at_t):
                """Transposes + agg (PE), gT/ag copies (ACT) for chunk c."""
                tr_ps = ps_tr.tile([P, 2 * NSL * P], FP16, space="PSUM",
                                   tag="tr")
                for k in range(NSL):
                    nc.tensor.transpose(
                        out=tr_ps[:, k * 2 * P:k * 2 * P + P],
                        in_=gsum[:, k * P:(k + 1) * P], identity=ident_t[:])
                    nc.tensor.transpose(
                        out=tr_ps[:, k * 2 * P + P:(k + 1) * 2 * P],
                        in_=gsq[:, k * P:(k + 1) * P], identity=ident_t[:])
                ag_ps = ps_ag.tile([P, NSL * P], FP16, space="PSUM", tag="ag")
                for k in range(NSL):
                    nc.tensor.transpose(
                        out=ag_ps[:, k * P:(k + 1) * P],
                        in_=at_t[:, k * P:(k + 1) * P], identity=ident_t[:])
                gT = gtp.tile([P, 2 * NSL * P], FP16, tag="gT")
                nc.scalar.copy(out=gT[:], in_=tr_ps[:])
                ag_t = agp.tile([P, NSL * P], FP16, tag="ag")
                nc.scalar.copy(out=ag_t[:], in_=ag_ps[:])

                seg_ps = ps_seg.tile([P, 2 * P], F32, space="PSUM", tag="seg")
                for k in range(NSL):
                    nc.tensor.matmul(
                        out=seg_ps[:],
                        lhsT=at_t[:, k * P:(k + 1) * P],
                        rhs=gT[:, k * 2 * P:(k + 1) * 2 * P],
                        start=(k == 0), stop=(k == NSL - 1),
                    )
                return seg_ps, ag_t

            def stage2(c, seg_ps, ag_t):
                """Stats for chunk c -> expand K,C to groups -> kcg sbuf."""
                rn_t = smallp.tile([P, 1], F32, tag="rn")
                nc.sync.dma_start(out=rn_t[:],
                                  in_=rn_d.ap()[c * P:(c + 1) * P, :])
                rn1_t = smallp.tile([P, 1], F32, tag="rn1")
                nc.sync.dma_start(out=rn1_t[:],
                                  in_=rn1_d.ap()[c * P:(c + 1) * P, :])
                mean_t = statp.tile([P, P], F32, tag="mean")
                nc.vector.tensor_scalar(
                    out=mean_t[:], in0=seg_ps[:, 0:P], scalar1=rn_t[:],
                    scalar2=None, op0=OP.mult,
                )
                t1 = statp.tile([P, P], F32, tag="t1")
                nc.vector.tensor_tensor(
                    out=t1[:], in0=mean_t[:], in1=seg_ps[:, 0:P], op=OP.mult)
                var_t = statp.tile([P, P], F32, tag="var")
                nc.vector.tensor_tensor(
                    out=var_t[:], in0=seg_ps[:, P:2 * P], in1=t1[:],
                    op=OP.subtract)
                nc.vector.tensor_scalar(
                    out=var_t[:], in0=var_t[:], scalar1=float(EPS),
                    scalar2=rn1_t[:], op0=OP.add, op1=OP.mult,
                )
                nc.vector.tensor_scalar(
                    out=var_t[:], in0=var_t[:], scalar1=float(VAR_FLOOR),
                    scalar2=None, op0=OP.max,
                )
                std_t = statp.tile([P, P], F32, tag="std")
                nc.scalar.activation(out=std_t[:], in_=var_t[:], func=AF.Sqrt)
                nc.scalar.activation(out=std_t[:], in_=std_t[:], func=AF.Copy,
                                     bias=float(EPS))
                kc32 = statp.tile([P, 2 * P], F32, tag="kc32")
                if fastwb:
                    nc.vector.reciprocal(out=kc32[:, 0:P], in_=std_t[:])
                    mk_t = statp.tile([P, P], F32, tag="mk")
                    nc.vector.tensor_tensor(
                        out=mk_t[:], in0=mean_t[:], in1=kc32[:, 0:P],
                        op=OP.mult)
                    nc.vector.tensor_scalar(
                        out=kc32[:, P:2 * P], in0=mk_t[:], scalar1=-1.0,
                        scalar2=None, op0=OP.mult)
                else:
                    rstd_t = statp.tile([P, P], F32, tag="rstd")
                    nc.vector.reciprocal(out=rstd_t[:], in_=std_t[:])
                    nc.vector.tensor_tensor(
                        out=kc32[:, 0:P], in0=rstd_t[:], in1=wb_t[:],
                        op=OP.mult)
                    mk_t = statp.tile([P, P], F32, tag="mk")
                    nc.vector.tensor_tensor(
                        out=mk_t[:], in0=mean_t[:], in1=kc32[:, 0:P],
                        op=OP.mult)
                    nc.vector.tensor_tensor(
                        out=kc32[:, P:2 * P], in0=bb_t[:], in1=mk_t[:],
                        op=OP.subtract)
                kc16 = statp.tile([P, 2 * P], FP16, tag="kc16")
                nc.scalar.copy(out=kc16[:], in_=kc32[:])

                kcg = kcp.tile([P, 2 * NG_C], FP16, tag="kcg")
                for half in range(2):
                    kg_ps = ps_kg.tile([P, NG_C], F32, space="PSUM", tag="kg")
                    src_t = kc16[:, half * P:(half + 1) * P]
                    for h in range(2):
                        nc.tensor.matmul(
                            out=kg_ps[:, h * 512:(h + 1) * 512],
                            lhsT=src_t, rhs=ag_t[:, h * 512:(h + 1) * 512],
                            start=True, stop=True,
                        )
                    nc.scalar.copy(
                        out=kcg[:, half * NG_C:(half + 1) * NG_C],
                        in_=kg_ps[:])
                return kcg

            def stage3(c, x_t, kcg):
                """Normalize chunk c: out = x*Kb + Cb, store."""
                x3 = x_t[:].rearrange("p (j g) -> p j g", g=NG_C)
                kb = kcg[:, 0:NG_C].rearrange("p (o g) -> p o g", o=1)
                kb = kb.broadcast_to([P, G, NG_C])
                cb = kcg[:, NG_C:2 * NG_C].rearrange("p (o g) -> p o g", o=1)
                cb = cb.broadcast_to([P, G, NG_C])
                y_t = sqp.tile([P, NA], FP16, tag="xsq")
                y3 = y_t[:].rearrange("p (j g) -> p j g", g=NG_C)
                nc.vector.tensor_tensor(out=y3, in0=x3, in1=kb, op=OP.mult)
                o_t = op_.tile([P, NA], FP16, tag="o")
                o3 = o_t[:].rearrange("p (j g) -> p j g", g=NG_C)
                nc.vector.tensor_tensor(out=o3, in0=y3, in1=cb, op=OP.add)
                nc.scalar.dma_start(out=out_d.ap()[c, :, :], in_=o_t[:])

            loaded = {0: load(0)}
            p1 = {}     # c -> (x_t, at_t, xsq, gsum)
            p3 = {}     # c -> (x_t, kcg)

            def mid(c):
                # pass2 + stats for chunk c (emitted at iter c+2)
                x_t, at_t, xsq, gsum = p1.pop(c)
                gsq = p1gsq.pop(c)
                seg_ps, ag_t = pass2(c, gsum, gsq, at_t)
                kcg = stage2(c, seg_ps, ag_t)
                p3[c] = (x_t, kcg)

            p1gsq = {}
            for c in range(nchunks):
                if c + 1 < nchunks:
                    loaded[c + 1] = load(c + 1)
                if c >= 3:
                    x_t, kcg = p3.pop(c - 3)
                    stage3(c - 3, x_t, kcg)
                x_t, at_t = loaded.pop(c)
                xsq, gsum = pass1a(c, x_t)
                p1[c] = (x_t, at_t, xsq, gsum)
                if c >= 2:
                    mid(c - 2)
                p1gsq[c] = pass1b(c, xsq)
            for c in range(nchunks - 2, nchunks):
                mid(c)
            for c in range(nchunks - 3, nchunks):
                if c >= 0:
                    x_t, kcg = p3.pop(c)
                    stage3(c, x_t, kcg)

    nc.compile()
    return nc


_BUILD_CACHE = {}


def kernel(target_fea, index, weight, bias):
    from concourse.bass_utils import run_bass_kernel_spmd

    x = np.asarray(target_fea, dtype=np.float32)
    idx = np.asarray(index, dtype=np.int64)
    xh = x.astype(np.float16)
    plans = _plan(idx)
    nchunks = max(len(p[0]) for p in plans)
    consts = _consts(weight, bias)

    cores = [_core_arrays(p, nchunks, xh) for p in plans]
    in_maps = []
    for ca in cores:
        m = {"xT": ca["xT"], "at": ca["at"], "rn": ca["rn"],
             "rn1": ca["rn1"]}
        m.update(consts)
        in_maps.append(m)

    fastwb = bool(np.all(np.asarray(weight) == 1.0)
                  and np.all(np.asarray(bias) == 0.0))
    key = (nchunks, fastwb)
    if key not in _BUILD_CACHE:
        _BUILD_CACHE[key] = _build(nchunks, fastwb)
    nc = _BUILD_CACHE[key]

    res = run_bass_kernel_spmd(nc, in_maps, core_ids=list(range(NCORES)))

    out = np.empty((N, F), dtype=np.float32)
    bias_np = np.asarray(bias, dtype=np.float32)
    for c in range(NCORES):
        ca = cores[c]
        r = np.asarray(res.results[c]["out"])
        atoms = np.ascontiguousarray(
            r.reshape(nchunks, F, G, NG_C).transpose(0, 3, 2, 1)
        ).reshape(nchunks * CHUNK_ATOMS, F)
        out[ca["src_rows"]] = atoms[ca["dst_rows"]].astype(np.float32)
        for d in ca["n1_dst"]:
            # n==1 segments: reference yields exactly bias
            src = ca["src_rows"][np.searchsorted(ca["dst_rows"], d)]
            out[src] = bias_np
    return out
